# revision 31
# baseline (speedup 1.0000x reference)
"""Tacotron-style location-sensitive attention on 8 TRN2 NeuronCores.

Data-parallel over batch: 64 examples -> 8 per core, weights replicated.
Per-core pipeline, all in layout A ([d_att/channel = partition, t = free]):
  1. p_lstm[a,b] = W_lstm @ Q^T + b_loc'       (TensorE, K=1024 in 8 chunks)
  2. arg[a,t]    = M^T @ im2col(aw_cum) + p_lstm + penc^T
                   where M = conv_w @ W_loc^T is host-folded (exact algebra),
                   per-partition p_lstm add + penc add fused in one DVE op
  3. e_all[b,t] += maskedWe[b]^T @ tanh(arg)   (TensorE; one row per example,
                   software-pipelined behind tanh to keep the PE FIFO moving)
  4. softmax: exp(e+b_e) with fused row-sum (ScalarE), reciprocal, scale (DVE)
  5. ctx[1,e]    = sum_t w[t] enc[t,e]         (TensorE, M=1, K=1024 chunks)

DMA streams are split to avoid sequencer head-of-line blocking:
  nc.sync   -> the 64 encoder-output tiles only (deep prefetch)
  nc.scalar -> constants, penc^T, im2col loads, and all output stores

Matmuls run in float32r (TF32 fast path, 1 cyc/row) — measured kernel-level
rel err ~1.6e-4. Host side does sharding + layout-only transforms + exact
weight folding (conv x W_loc product, bias folding).
"""

import numpy as np

B, T, E, D_LSTM, D_ATT, C_LOC, K = 64, 1024, 512, 1024, 128, 32, 31
N_CORES = 8
BL = B // N_CORES  # examples per core
TC = T // 128      # t-chunks of 128
PAD = K // 2

ENC_BUFS = 38      # deep prefetch of encoder tiles (2 KB/partition each)
PIPE_DEPTH = 3     # masked-We matmul trails tanh by this many iterations

_cache = {}


def _build_nc():
    import concourse.bacc as bacc
    import concourse.mybir as mybir
    from concourse import tile
    from concourse.tile import add_dep_helper
    from concourse.ap import AP
    from contextlib import ExitStack

    dt = mybir.dt.float32
    dtr = mybir.dt.float32r
    Alu = mybir.AluOpType
    Act = mybir.ActivationFunctionType

    nc = bacc.Bacc(target_bir_lowering=False)

    enc = nc.declare_dram_parameter("enc", [BL, T, E], dtr, isOutput=False)
    pencT = nc.declare_dram_parameter("pencT", [BL, 2, 128, T // 2], dt, isOutput=False)
    xcol_h = nc.declare_dram_parameter("xcol_h", [BL, K, T], dtr, isOutput=False)
    blob_r = nc.declare_dram_parameter("blob_r", [128, 1536], dtr, isOutput=False)
    blob_f = nc.declare_dram_parameter("blob_f", [128, 16], dt, isOutput=False)
    out_ctx = nc.declare_dram_parameter("out_ctx", [BL, E], dt, isOutput=True)
    out_w = nc.declare_dram_parameter("out_w", [BL, T], dt, isOutput=True)

    with ExitStack() as ctx:
        tc = ctx.enter_context(tile.TileContext(nc))

        const = ctx.enter_context(tc.tile_pool(name="const", bufs=1))
        penc_pool = ctx.enter_context(tc.tile_pool(name="penc", bufs=BL))
        xcol_pool = ctx.enter_context(tc.tile_pool(name="xcol", bufs=BL))
        targ_pool = ctx.enter_context(tc.tile_pool(name="targ", bufs=4))
        tanh_pool = ctx.enter_context(tc.tile_pool(name="tanh", bufs=PIPE_DEPTH + 3))
        soft_pool = ctx.enter_context(tc.tile_pool(name="soft", bufs=1))
        wt_pool = ctx.enter_context(tc.tile_pool(name="wt", bufs=TC))
        enc_pool = ctx.enter_context(tc.tile_pool(name="encp", bufs=ENC_BUFS))
        cout_pool = ctx.enter_context(tc.tile_pool(name="cout", bufs=1))

        # Two independent DMA pipes, each FIFO-ordered so its own phase-1
        # inputs precede its enc stream; the scalar ring carries no DMAs so
        # ACT compute is never blocked behind descriptor generation.
        #   sync  (HWDGE): blobs, xcol/penc b0-3, enc b0-3, out_ctx
        #   gpsimd(SWDGE): xcol/penc b4-7, enc b4-7, out_w
        blob_r_sb = const.tile([128, 1536], dtr, tag="blob_r")
        nc.sync.dma_start(out=blob_r_sb[:], in_=blob_r[:, :])
        blob_f_sb = const.tile([128, 16], dt, tag="blob_f")
        nc.sync.dma_start(out=blob_f_sb[:], in_=blob_f[:, :])
        wemask_sb = blob_r_sb[:, 0:64]
        lstmT_sb = blob_r_sb[:, 64:128]
        m31_sb = blob_r_sb[0:K, 128:256]
        blocr_sb = blob_r_sb[0:1, 256:384]
        ones8_sb = blob_r_sb[0:1, 384:392]
        wl_sb = blob_r_sb[:, 512:1536]
        bebc_sb = blob_f_sb[0:BL, 0:1]
        ident_sb = blob_f_sb[0:BL, 8:16]

        xcol_sb = []
        pencT_sb = []
        for b in range(BL):
            eng = nc.sync if b < 4 else nc.gpsimd
            xcol = xcol_pool.tile([K, T], dtr, tag="xcol")
            eng.dma_start(out=xcol[:], in_=xcol_h[b, :, :])
            xcol_sb.append(xcol)
            pp = penc_pool.tile([128, T], dt, tag="pt")
            eng.dma_start(out=pp[:, 0:T // 2], in_=pencT[b, 0, :, :])
            eng.dma_start(out=pp[:, T // 2:], in_=pencT[b, 1, :, :])
            pencT_sb.append(pp)

        # encoder tiles: one contiguous 1 MB DMA per half-example.
        # b0-3 stream on the sync ring (behind xcol+penc = FIFO priority),
        # b4-7 on the gpsimd SWDGE path so two DMA pipes run in parallel
        # and the scalar ring stays free for ACT compute.
        enc_sb = [None] * (BL * TC)
        stt_insts = []

        exp_sb = soft_pool.tile([BL, T], dt, tag="exp_sb")
        sums = soft_pool.tile([BL, 1], dt, tag="sums")
        inv = soft_pool.tile([BL, 1], dt, tag="inv")
        w_norm = soft_pool.tile([BL, T], dt, tag="w_norm")
        p_sb = soft_pool.tile([D_ATT, BL], dt, tag="p_sb")

        with tc.tile_pool(name="pse", bufs=1, space="PSUM") as pe_pool:
            e_all = pe_pool.tile([BL, T], dt, tag="e_all")

            # p_lstm: [128a, 8b] = W_lstm @ Q^T  (+ b_loc' x ones)
            with tc.tile_pool(name="ps1", bufs=1, space="PSUM") as ps1:
                p_ps = ps1.tile([D_ATT, BL], dt, tag="p_ps")
                for c in range(TC):
                    nc.tensor.matmul(
                        p_ps[:],
                        wl_sb[:, c * 128:(c + 1) * 128],
                        lstmT_sb[:, c * BL:(c + 1) * BL],
                        start=(c == 0), stop=False,
                    )
                nc.tensor.matmul(
                    p_ps[:], blocr_sb, ones8_sb,
                    start=False, stop=True,
                )
                nc.vector.tensor_copy(p_sb[:], p_ps[:])

            # energies, software-pipelined
            with tc.tile_pool(name="pst", bufs=4, space="PSUM") as pst:
                pend = []

                def flush_one():
                    b, h, th = pend.pop(0)
                    nc.tensor.matmul(
                        e_all[:, h * 512:(h + 1) * 512],
                        wemask_sb[:, b * BL:(b + 1) * BL],
                        th[:],
                        start=(b == 0), stop=(b == BL - 1),
                        skip_group_check=True,
                    )

                for b in range(BL):
                    if b == 5:
                        for b2 in range(5, BL):
                            for c in range(TC):
                                et = enc_pool.tile([128, E], dtr, tag="et")
                                nc.scalar.dma_start(
                                    out=et[:],
                                    in_=enc[b2, c * 128:(c + 1) * 128, :],
                                )
                                enc_sb[b2 * TC + c] = et
                    for h in range(2):
                        tps = pst.tile([D_ATT, 512], dt, tag="tps")
                        nc.tensor.matmul(
                            tps[:],
                            m31_sb,
                            xcol_sb[b][:, h * 512:(h + 1) * 512],
                            start=True, stop=True,
                        )
                        targ = targ_pool.tile([D_ATT, 512], dt, tag="targ")
                        stt_insts.append(nc.vector.scalar_tensor_tensor(
                            out=targ[:],
                            in0=tps[:],
                            scalar=p_sb[:, b:b + 1],
                            in1=pencT_sb[b][:, h * 512:(h + 1) * 512],
                            op0=Alu.add,
                            op1=Alu.add,
                        ))
                        th = tanh_pool.tile([D_ATT, 512], dtr, tag="th")
                        nc.scalar.activation(th[:], targ[:], Act.Tanh)
                        pend.append((b, h, th))
                        if len(pend) > PIPE_DEPTH:
                            flush_one()
                while pend:
                    flush_one()

            # encoder stream three ways: sync b0-2, gpsimd b3-5 here (gated
            # on energy progress so phase-1 input retirement is undisturbed);
            # b6-7 follow on the scalar ring after the softmax trace point.
            for b in range(5):
                for c in range(TC):
                    et = enc_pool.tile([128, E], dtr, tag="et")
                    eng = nc.sync if b < 3 else nc.gpsimd
                    eng.dma_start(
                        out=et[:], in_=enc[b, c * 128:(c + 1) * 128, :]
                    )
                    enc_sb[b * TC + c] = et

            # softmax (no max-subtract: |e| <= ~5.2 by construction)
            nc.scalar.activation(
                exp_sb[:], e_all[:], Act.Exp,
                bias=bebc_sb, accum_out=sums[:],
            )

        nc.vector.reciprocal(inv[:], sums[:])
        nc.vector.tensor_scalar_mul(w_norm[:], exp_sb[:], inv[:])

        nc.gpsimd.dma_start(out=out_w[:, :], in_=w_norm[:])

        # ---------- phase 2: context ----------
        with tc.tile_pool(name="pswt", bufs=2, space="PSUM") as ps_wt, \
             tc.tile_pool(name="psctx", bufs=2, space="PSUM") as ps_ctx:

            wt_sb = []
            for c in range(TC):
                wps = ps_wt.tile([128, BL], dt, tag="wps")
                nc.tensor.transpose(
                    wps[:], w_norm[:, c * 128:(c + 1) * 128], ident_sb
                )
                wsb = wt_pool.tile([128, BL], dtr, tag="wsb")
                nc.vector.tensor_copy(wsb[:], wps[:])
                wt_sb.append(wsb)

            ctx_all = cout_pool.tile([1, BL * E], dt, tag="ctx_all")
            for b in range(BL):
                cxp = ps_ctx.tile([1, E], dt, tag="cxp")
                for c in range(TC):
                    nc.tensor.matmul(
                        cxp[:],
                        wt_sb[c][:, b:b + 1],
                        enc_sb[b * TC + c][:],
                        start=(c == 0), stop=(c == TC - 1),
                    )
                nc.vector.tensor_copy(ctx_all[:, b * E:(b + 1) * E], cxp[:])
                if b == 3:
                    nc.sync.dma_start(
                        out=AP(out_ctx, 0, [[1, 1], [1, 4 * E]]),
                        in_=ctx_all[:, 0:4 * E],
                    )
            nc.sync.dma_start(
                out=AP(out_ctx, 4 * E, [[1, 1], [1, 4 * E]]),
                in_=ctx_all[:, 4 * E:],
            )

    nc.finalize()
    return nc


def _shard(inp, i):
    f32 = np.float32
    sl = slice(i * BL, (i + 1) * BL)
    enc = np.ascontiguousarray(np.asarray(inp["encoder_output"])[sl], f32)
    pencT = np.ascontiguousarray(
        np.asarray(inp["processed_encoder_output"])[sl].transpose(0, 2, 1)
        .reshape(BL, 128, 2, T // 2).transpose(0, 2, 1, 3), f32
    )
    Q = np.asarray(inp["lstm_output"], f32)[sl, 0, :]  # [BL, D_LSTM]
    lstmT = np.ascontiguousarray(
        Q.T.reshape(TC, 128, BL).transpose(1, 0, 2).reshape(128, TC * BL), f32
    )
    awcp = np.pad(
        np.asarray(inp["attention_weights_cum"], f32)[sl], ((0, 0), (PAD, PAD))
    )
    # host im2col (pure gather): xcol_h[b, k, t] = awcp[b, k + t]
    xcol_h = np.ascontiguousarray(
        np.lib.stride_tricks.sliding_window_view(awcp, T, axis=1)
        .transpose(0, 1, 2)[:, 0:K, :]
    )
    wlstmT = np.ascontiguousarray(
        np.asarray(inp["W_lstm"], f32).T.reshape(TC, 128, D_ATT)
        .transpose(1, 0, 2).reshape(128, D_LSTM)
    )
    conv_w = np.asarray(inp["conv_w"], f32)[:, 0, :]      # [C_LOC, K]
    conv_b = np.asarray(inp["conv_b"], f32)               # [C_LOC]
    W_loc = np.asarray(inp["W_loc"], f32)                 # [D_ATT, C_LOC]
    # exact algebraic folding: loc-conv then W_loc projection == one matmul
    m31 = conv_w.T @ W_loc.T                              # [K, D_ATT]
    blocr = np.asarray(inp["b_loc"], f32) + W_loc @ conv_b  # [D_ATT]
    we = np.asarray(inp["W_e"], f32).reshape(D_ATT)

    blob_r = np.zeros((128, 1536), f32)
    for b in range(BL):
        blob_r[:, b * BL + b] = we                        # wemask cols 0:64
    blob_r[:, 64:128] = lstmT
    blob_r[0:K, 128:256] = m31
    blob_r[0, 256:384] = blocr
    blob_r[0, 384:392] = 1.0                              # ones8
    blob_r[:, 512:1536] = wlstmT
    blob_f = np.zeros((128, 16), f32)
    blob_f[0:BL, 0] = np.asarray(inp["b_e"], f32).reshape(-1)[0]
    blob_f[0:BL, 8:16] = np.eye(BL, dtype=f32)
    return dict(enc=enc, pencT=pencT, xcol_h=xcol_h, blob_r=blob_r,
                blob_f=blob_f)


def _run(inputs, trace=False):
    from concourse.bass_utils import run_bass_kernel_spmd

    if "nc" not in _cache:
        _cache["nc"] = _build_nc()
    in_maps = [_shard(inputs, i) for i in range(N_CORES)]
    res = run_bass_kernel_spmd(
        _cache["nc"], in_maps, core_ids=list(range(N_CORES)), trace=trace
    )
    ctx = np.concatenate([r["out_ctx"] for r in res.results], axis=0)
    w = np.concatenate([r["out_w"] for r in res.results], axis=0)
    out = (
        np.asarray(ctx, np.float32).reshape(B, 1, E),
        np.asarray(w, np.float32).reshape(B, T),
    )
    return out, res


def kernel(**inputs):
    out, _ = _run(inputs, trace=False)
    return out


def kernel_traced(**inputs):
    out, res = _run(inputs, trace=True)
    return out, res


# revision 32
# speedup vs baseline: 1.1249x; 1.1249x over previous
"""Tacotron-style location-sensitive attention on 8 TRN2 NeuronCores.

Data-parallel over batch: 64 examples -> 8 per core, weights replicated.
Per-core pipeline, all in layout A ([d_att/channel = partition, t = free]):
  1. p_lstm[a,b] = W_lstm @ Q^T + b_loc'       (TensorE, K=1024 in 8 chunks)
  2. arg[a,t]    = M^T @ im2col(aw_cum) + p_lstm + penc^T
                   where M = conv_w @ W_loc^T is host-folded (exact algebra),
                   per-partition p_lstm add + penc add fused in one DVE op
  3. e_all[b,t] += maskedWe[b]^T @ tanh(arg)   (TensorE; one row per example,
                   software-pipelined behind tanh to keep the PE FIFO moving)
  4. softmax: exp(e+b_e) with fused row-sum (ScalarE), reciprocal, scale (DVE)
  5. ctx[1,e]    = sum_t w[t] enc[t,e]         (TensorE, M=1, K=1024 chunks)

DMA streams are split to avoid sequencer head-of-line blocking:
  nc.sync   -> the 64 encoder-output tiles only (deep prefetch)
  nc.scalar -> constants, penc^T, im2col loads, and all output stores

Matmuls run in float32r (TF32 fast path, 1 cyc/row) — measured kernel-level
rel err ~1.6e-4. Host side does sharding + layout-only transforms + exact
weight folding (conv x W_loc product, bias folding).
"""

import numpy as np

B, T, E, D_LSTM, D_ATT, C_LOC, K = 64, 1024, 512, 1024, 128, 32, 31
N_CORES = 8
BL = B // N_CORES  # examples per core
TC = T // 128      # t-chunks of 128
PAD = K // 2

ENC_BUFS = 38      # deep prefetch of encoder tiles (2 KB/partition each)
PIPE_DEPTH = 3     # masked-We matmul trails tanh by this many iterations

_cache = {}


def _build_nc():
    import concourse.bacc as bacc
    import concourse.mybir as mybir
    from concourse import tile
    from concourse.tile import add_dep_helper
    from concourse.ap import AP
    from contextlib import ExitStack

    dt = mybir.dt.float32
    dtr = mybir.dt.float32r
    Alu = mybir.AluOpType
    Act = mybir.ActivationFunctionType

    nc = bacc.Bacc(target_bir_lowering=False)

    enc = nc.declare_dram_parameter("enc", [BL, T, E], dtr, isOutput=False)
    pencT = nc.declare_dram_parameter("pencT", [BL, 2, 128, T // 2], dt, isOutput=False)
    xcol_h = nc.declare_dram_parameter("xcol_h", [BL, K, T], dtr, isOutput=False)
    blob_r = nc.declare_dram_parameter("blob_r", [128, 1536], dtr, isOutput=False)
    blob_f = nc.declare_dram_parameter("blob_f", [128, 16], dt, isOutput=False)
    out_ctx = nc.declare_dram_parameter("out_ctx", [BL, E], dt, isOutput=True)
    out_w = nc.declare_dram_parameter("out_w", [BL, T], dt, isOutput=True)

    with ExitStack() as ctx:
        tc = ctx.enter_context(tile.TileContext(nc))

        const = ctx.enter_context(tc.tile_pool(name="const", bufs=1))
        penc_pool = ctx.enter_context(tc.tile_pool(name="penc", bufs=BL))
        xcol_pool = ctx.enter_context(tc.tile_pool(name="xcol", bufs=BL))
        targ_pool = ctx.enter_context(tc.tile_pool(name="targ", bufs=4))
        tanh_pool = ctx.enter_context(tc.tile_pool(name="tanh", bufs=PIPE_DEPTH + 3))
        soft_pool = ctx.enter_context(tc.tile_pool(name="soft", bufs=1))
        wt_pool = ctx.enter_context(tc.tile_pool(name="wt", bufs=TC))
        enc_pool = ctx.enter_context(tc.tile_pool(name="encp", bufs=ENC_BUFS))
        cout_pool = ctx.enter_context(tc.tile_pool(name="cout", bufs=1))

        # Two independent DMA pipes, each FIFO-ordered so its own phase-1
        # inputs precede its enc stream; the scalar ring carries no DMAs so
        # ACT compute is never blocked behind descriptor generation.
        #   sync  (HWDGE): blobs, xcol/penc b0-3, enc b0-3, out_ctx
        #   gpsimd(SWDGE): xcol/penc b4-7, enc b4-7, out_w
        blob_r_sb = const.tile([128, 1536], dtr, tag="blob_r")
        nc.sync.dma_start(out=blob_r_sb[:], in_=blob_r[:, :])
        blob_f_sb = const.tile([128, 16], dt, tag="blob_f")
        nc.sync.dma_start(out=blob_f_sb[:], in_=blob_f[:, :])
        wemask_sb = blob_r_sb[:, 0:64]
        lstmT_sb = blob_r_sb[:, 64:128]
        m31_sb = blob_r_sb[0:K, 128:256]
        blocr_sb = blob_r_sb[0:1, 256:384]
        ones8_sb = blob_r_sb[0:1, 384:392]
        wl_sb = blob_r_sb[:, 512:1536]
        bebc_sb = blob_f_sb[0:BL, 0:1]
        ident_sb = blob_f_sb[0:BL, 8:16]

        xcol_sb = []
        pencT_sb = []
        for b in range(BL):
            eng = nc.sync if b < 4 else nc.gpsimd
            xcol = xcol_pool.tile([K, T], dtr, tag="xcol")
            eng.dma_start(out=xcol[:], in_=xcol_h[b, :, :])
            xcol_sb.append(xcol)
            pp = penc_pool.tile([128, T], dt, tag="pt")
            eng.dma_start(out=pp[:, 0:T // 2], in_=pencT[b, 0, :, :])
            eng.dma_start(out=pp[:, T // 2:], in_=pencT[b, 1, :, :])
            pencT_sb.append(pp)

        # encoder tiles: one contiguous 1 MB DMA per half-example.
        # b0-3 stream on the sync ring (behind xcol+penc = FIFO priority),
        # b4-7 on the gpsimd SWDGE path so two DMA pipes run in parallel
        # and the scalar ring stays free for ACT compute.
        enc_sb = [None] * (BL * TC)
        stt_insts = []

        exp_sb = soft_pool.tile([BL, T], dt, tag="exp_sb")
        sums = soft_pool.tile([BL, 1], dt, tag="sums")
        inv = soft_pool.tile([BL, 1], dt, tag="inv")
        w_norm = soft_pool.tile([BL, T], dt, tag="w_norm")
        p_sb = soft_pool.tile([D_ATT, BL], dt, tag="p_sb")

        with tc.tile_pool(name="pse", bufs=1, space="PSUM") as pe_pool:
            e_all = pe_pool.tile([BL, T], dt, tag="e_all")

            # p_lstm: [128a, 8b] = W_lstm @ Q^T  (+ b_loc' x ones)
            with tc.tile_pool(name="ps1", bufs=1, space="PSUM") as ps1:
                p_ps = ps1.tile([D_ATT, BL], dt, tag="p_ps")
                for c in range(TC):
                    nc.tensor.matmul(
                        p_ps[:],
                        wl_sb[:, c * 128:(c + 1) * 128],
                        lstmT_sb[:, c * BL:(c + 1) * BL],
                        start=(c == 0), stop=False,
                    )
                nc.tensor.matmul(
                    p_ps[:], blocr_sb, ones8_sb,
                    start=False, stop=True,
                )
                nc.vector.tensor_copy(p_sb[:], p_ps[:])

            # energies, software-pipelined
            with tc.tile_pool(name="pst", bufs=4, space="PSUM") as pst:
                pend = []

                def flush_one():
                    b, h, th = pend.pop(0)
                    nc.tensor.matmul(
                        e_all[:, h * 512:(h + 1) * 512],
                        wemask_sb[:, b * BL:(b + 1) * BL],
                        th[:],
                        start=(b == 0), stop=(b == BL - 1),
                        skip_group_check=True,
                    )

                for b in range(BL):
                    if b == 5:
                        for b2 in range(6, BL):
                            for c in range(TC):
                                et = enc_pool.tile([128, E], dtr, tag="et")
                                nc.scalar.dma_start(
                                    out=et[:],
                                    in_=enc[b2, c * 128:(c + 1) * 128, :],
                                )
                                enc_sb[b2 * TC + c] = et
                    for h in range(2):
                        tps = pst.tile([D_ATT, 512], dt, tag="tps")
                        nc.tensor.matmul(
                            tps[:],
                            m31_sb,
                            xcol_sb[b][:, h * 512:(h + 1) * 512],
                            start=True, stop=True,
                        )
                        targ = targ_pool.tile([D_ATT, 512], dt, tag="targ")
                        stt_insts.append(nc.vector.scalar_tensor_tensor(
                            out=targ[:],
                            in0=tps[:],
                            scalar=p_sb[:, b:b + 1],
                            in1=pencT_sb[b][:, h * 512:(h + 1) * 512],
                            op0=Alu.add,
                            op1=Alu.add,
                        ))
                        th = tanh_pool.tile([D_ATT, 512], dtr, tag="th")
                        nc.scalar.activation(th[:], targ[:], Act.Tanh)
                        pend.append((b, h, th))
                        if len(pend) > PIPE_DEPTH:
                            flush_one()
                while pend:
                    flush_one()

            # encoder stream three ways: sync b0-2, gpsimd b3-5 here (gated
            # on energy progress so phase-1 input retirement is undisturbed);
            # b6-7 follow on the scalar ring after the softmax trace point.
            for b in range(6):
                for c in range(TC):
                    et = enc_pool.tile([128, E], dtr, tag="et")
                    eng = nc.sync if b < 3 else nc.gpsimd
                    eng.dma_start(
                        out=et[:], in_=enc[b, c * 128:(c + 1) * 128, :]
                    )
                    enc_sb[b * TC + c] = et

            # softmax (no max-subtract: |e| <= ~5.2 by construction)
            nc.scalar.activation(
                exp_sb[:], e_all[:], Act.Exp,
                bias=bebc_sb, accum_out=sums[:],
            )

        nc.vector.reciprocal(inv[:], sums[:])
        nc.vector.tensor_scalar_mul(w_norm[:], exp_sb[:], inv[:])

        nc.gpsimd.dma_start(out=out_w[:, :], in_=w_norm[:])

        # ---------- phase 2: context ----------
        with tc.tile_pool(name="pswt", bufs=2, space="PSUM") as ps_wt, \
             tc.tile_pool(name="psctx", bufs=2, space="PSUM") as ps_ctx:

            wt_sb = []
            for c in range(TC):
                wps = ps_wt.tile([128, BL], dt, tag="wps")
                nc.tensor.transpose(
                    wps[:], w_norm[:, c * 128:(c + 1) * 128], ident_sb
                )
                wsb = wt_pool.tile([128, BL], dtr, tag="wsb")
                nc.vector.tensor_copy(wsb[:], wps[:])
                wt_sb.append(wsb)

            ctx_all = cout_pool.tile([1, BL * E], dt, tag="ctx_all")
            for b in range(BL):
                cxp = ps_ctx.tile([1, E], dt, tag="cxp")
                for c in range(TC):
                    nc.tensor.matmul(
                        cxp[:],
                        wt_sb[c][:, b:b + 1],
                        enc_sb[b * TC + c][:],
                        start=(c == 0), stop=(c == TC - 1),
                    )
                nc.vector.tensor_copy(ctx_all[:, b * E:(b + 1) * E], cxp[:])
                if b == 3:
                    nc.sync.dma_start(
                        out=AP(out_ctx, 0, [[1, 1], [1, 4 * E]]),
                        in_=ctx_all[:, 0:4 * E],
                    )
            nc.sync.dma_start(
                out=AP(out_ctx, 4 * E, [[1, 1], [1, 4 * E]]),
                in_=ctx_all[:, 4 * E:],
            )

    nc.finalize()
    return nc


def _shard(inp, i):
    f32 = np.float32
    sl = slice(i * BL, (i + 1) * BL)
    enc = np.ascontiguousarray(np.asarray(inp["encoder_output"])[sl], f32)
    pencT = np.ascontiguousarray(
        np.asarray(inp["processed_encoder_output"])[sl].transpose(0, 2, 1)
        .reshape(BL, 128, 2, T // 2).transpose(0, 2, 1, 3), f32
    )
    Q = np.asarray(inp["lstm_output"], f32)[sl, 0, :]  # [BL, D_LSTM]
    lstmT = np.ascontiguousarray(
        Q.T.reshape(TC, 128, BL).transpose(1, 0, 2).reshape(128, TC * BL), f32
    )
    awcp = np.pad(
        np.asarray(inp["attention_weights_cum"], f32)[sl], ((0, 0), (PAD, PAD))
    )
    # host im2col (pure gather): xcol_h[b, k, t] = awcp[b, k + t]
    xcol_h = np.ascontiguousarray(
        np.lib.stride_tricks.sliding_window_view(awcp, T, axis=1)
        .transpose(0, 1, 2)[:, 0:K, :]
    )
    wlstmT = np.ascontiguousarray(
        np.asarray(inp["W_lstm"], f32).T.reshape(TC, 128, D_ATT)
        .transpose(1, 0, 2).reshape(128, D_LSTM)
    )
    conv_w = np.asarray(inp["conv_w"], f32)[:, 0, :]      # [C_LOC, K]
    conv_b = np.asarray(inp["conv_b"], f32)               # [C_LOC]
    W_loc = np.asarray(inp["W_loc"], f32)                 # [D_ATT, C_LOC]
    # exact algebraic folding: loc-conv then W_loc projection == one matmul
    m31 = conv_w.T @ W_loc.T                              # [K, D_ATT]
    blocr = np.asarray(inp["b_loc"], f32) + W_loc @ conv_b  # [D_ATT]
    we = np.asarray(inp["W_e"], f32).reshape(D_ATT)

    blob_r = np.zeros((128, 1536), f32)
    for b in range(BL):
        blob_r[:, b * BL + b] = we                        # wemask cols 0:64
    blob_r[:, 64:128] = lstmT
    blob_r[0:K, 128:256] = m31
    blob_r[0, 256:384] = blocr
    blob_r[0, 384:392] = 1.0                              # ones8
    blob_r[:, 512:1536] = wlstmT
    blob_f = np.zeros((128, 16), f32)
    blob_f[0:BL, 0] = np.asarray(inp["b_e"], f32).reshape(-1)[0]
    blob_f[0:BL, 8:16] = np.eye(BL, dtype=f32)
    return dict(enc=enc, pencT=pencT, xcol_h=xcol_h, blob_r=blob_r,
                blob_f=blob_f)


def _run(inputs, trace=False):
    from concourse.bass_utils import run_bass_kernel_spmd

    if "nc" not in _cache:
        _cache["nc"] = _build_nc()
    in_maps = [_shard(inputs, i) for i in range(N_CORES)]
    res = run_bass_kernel_spmd(
        _cache["nc"], in_maps, core_ids=list(range(N_CORES)), trace=trace
    )
    ctx = np.concatenate([r["out_ctx"] for r in res.results], axis=0)
    w = np.concatenate([r["out_w"] for r in res.results], axis=0)
    out = (
        np.asarray(ctx, np.float32).reshape(B, 1, E),
        np.asarray(w, np.float32).reshape(B, T),
    )
    return out, res


def kernel(**inputs):
    out, _ = _run(inputs, trace=False)
    return out


def kernel_traced(**inputs):
    out, res = _run(inputs, trace=True)
    return out, res
